# revision 1
# baseline (speedup 1.0000x reference)
"""Causal self-attention (B=4, T=2048, D=1024, H=16, DH=64) on 8 TRN2 NeuronCores.

Sharding: core c handles batch b = c//2 and head group hg = (c%2)*8 (8 of 16
heads), Megatron-style on the head dim. Each core computes QKV for its heads,
causal attention, and its partial output projection; the host sums the two
partial projections per batch.

On-chip layout (per core):
  - qkv computed transposed: q^T/k^T as [feat(128-part), tok] tiles, v in
    natural [tok, feat] layout with an appended ones column so the PV matmul
    also produces the softmax normalizer l.
  - softmax without max-subtraction (scores ~ N(0,1): exp never overflows);
    causal masking by multiplying exp tiles with 0/1 masks on diagonal blocks,
    fully-masked blocks are skipped.
  - all matmuls in float32r (full PE rate at N=512, ~tf32 precision).
"""
import sys
import types

import numpy as np

# If the image lacks antenv.axon_hooks, register a compatible stub so
# run_bass_kernel_spmd(trace=True)/BASS_TRACE=1 can capture NTFF profiles
# (falls back to no-op when the axon client library has no profile export).
try:
    import antenv.axon_hooks  # noqa: F401
except ImportError:
    try:
        from trn_agent_boot.trn_boot import _ntff_profile_via_ctypes

        _hook = _ntff_profile_via_ctypes("/opt/axon/libaxon_pjrt.so")
    except Exception:
        _hook = None
    _m = types.ModuleType("antenv.axon_hooks")
    _m.get_axon_ntff_profile_hook = lambda: _hook
    _m.set_axon_ntff_profile_hook = lambda h: None
    sys.modules["antenv.axon_hooks"] = _m

import concourse.bass_utils as _bass_utils

if getattr(_bass_utils, "_local_artifacts_patch", None) is None:
    _bass_utils.upload_artifacts = lambda tmpdir: tmpdir
    _bass_utils._local_artifacts_patch = True

import concourse.bacc as bacc
import concourse.tile as tile
from concourse import mybir
from concourse.bass_utils import run_bass_kernel_spmd

F32 = mybir.dt.float32
F32R = mybir.dt.float32r
EXP = mybir.ActivationFunctionType.Exp

B, T, D = 4, 2048, 1024
H, DH = 16, 64
HPC = 8             # heads per core
P = 128
NSLAB = T // 512    # 4 query slabs
DC = D // P         # 8 d-chunks
N_CORES = 8

_cached_nc = None
LAST_EXEC_NS = None


def _build_program():
    nc = bacc.Bacc("TRN2", target_bir_lowering=False, debug=False, num_devices=N_CORES)
    xt_d = nc.dram_tensor("xt", [D, T], F32R, kind="ExternalInput").ap()
    wqk_d = nc.dram_tensor("wqk", [D, 2 * HPC * DH], F32R, kind="ExternalInput").ap()
    wv_d = nc.dram_tensor("wv", [D, HPC * DH], F32R, kind="ExternalInput").ap()
    wp_d = nc.dram_tensor("wp", [HPC * DH, D], F32R, kind="ExternalInput").ap()
    masks_d = nc.dram_tensor("masks", [P, 4, 512], F32R, kind="ExternalInput").ap()
    out_d = nc.dram_tensor("out", [T, D], F32, kind="ExternalOutput").ap()

    with tile.TileContext(nc) as tc:
        lp = nc.allow_low_precision(reason="fp32r matmul inputs")
        lp.__enter__()
        with (
            tc.tile_pool(name="persist", bufs=1) as persist,
            tc.tile_pool(name="small", bufs=1) as small,
        ):
            masks = persist.tile([P, 4, 512], F32R)
            nc.sync.dma_start(masks, masks_d)
            # q^T/k^T feature tiles: f in 0..3 -> q feats 128f..;  4..7 -> k
            qk = persist.tile([P, 8, T], F32R)
            # v natural layout + ones column: [tok-tile, head, dh+1]
            vt = persist.tile([P, T // P, HPC, DH + 1], F32R)
            ones_f = small.tile([P, (T // P) * HPC], F32)
            nc.vector.memset(ones_f, 1.0)
            nc.vector.tensor_copy(
                vt[:, :, :, DH : DH + 1],
                ones_f.rearrange("p (a b) -> p a b", a=T // P).unsqueeze(3),
            )
            r_all = small.tile([P, 512], F32)

            # ---- Phase 1: QKV projections ----
            with (
                tc.tile_pool(name="xs", bufs=2) as xpool,
                tc.tile_pool(name="w1", bufs=1) as wpool,
                tc.tile_pool(name="qkps", bufs=3, space="PSUM") as qkps,
                tc.tile_pool(name="vps", bufs=2, space="PSUM") as vps,
            ):
                wqk = wpool.tile([P, DC, 2 * HPC * DH], F32R)
                wv = wpool.tile([P, DC, HPC * DH], F32R)
                for c in range(DC):
                    nc.sync.dma_start(wqk[:, c, :], wqk_d[P * c : P * (c + 1), :])
                    nc.sync.dma_start(wv[:, c, :], wv_d[P * c : P * (c + 1), :])
                for j in range(NSLAB):
                    xs = xpool.tile([P, DC, 512], F32R, tag="xs")
                    for c in range(DC):
                        nc.sync.dma_start(
                            xs[:, c, :], xt_d[P * c : P * (c + 1), 512 * j : 512 * (j + 1)]
                        )
                    for f in range(8):
                        ps = qkps.tile([P, 512], F32, tag="qk")
                        for c in range(DC):
                            nc.tensor.matmul(
                                ps,
                                wqk[:, c, P * f : P * (f + 1)],
                                xs[:, c, :],
                                start=(c == 0),
                                stop=(c == DC - 1),
                            )
                        nc.vector.tensor_copy(qk[:, f, 512 * j : 512 * (j + 1)], ps)
                    for tt in range(4):
                        psv = vps.tile([P, 512], F32, tag="v")
                        for c in range(DC):
                            nc.tensor.matmul(
                                psv,
                                xs[:, c, P * tt : P * (tt + 1)],
                                wv[:, c, :],
                                start=(c == 0),
                                stop=(c == DC - 1),
                            )
                        nc.vector.tensor_copy(
                            vt[:, 4 * j + tt, :, 0:DH],
                            psv.rearrange("p (h d) -> p h d", h=HPC),
                        )

            # ---- Phase 2: attention + output projection, per query slab ----
            with (
                tc.tile_pool(name="yt", bufs=1) as ytpool,
                tc.tile_pool(name="w2", bufs=1) as w2pool,
                tc.tile_pool(name="pp", bufs=4) as ppool,
                tc.tile_pool(name="tails", bufs=2) as tails,
                tc.tile_pool(name="outsb", bufs=3) as outsb,
                tc.tile_pool(name="sps", bufs=3, space="PSUM") as sps,
                tc.tile_pool(name="pvps", bufs=2, space="PSUM") as pvps,
                tc.tile_pool(name="projps", bufs=2, space="PSUM") as projps,
            ):
                # y^T packed: chunk c rows 0..63 head 2c, 64..127 head 2c+1
                yt = ytpool.tile([P, HPC // 2, T], F32R)
                wp = w2pool.tile([P, HPC * DH // P, D], F32R)
                for c in range(HPC * DH // P):
                    nc.sync.dma_start(wp[:, c, :], wp_d[P * c : P * (c + 1), :])
                for j in range(NSLAB):
                    qs = slice(512 * j, 512 * (j + 1))
                    kmax = 4 * j + 4
                    for h in range(HPC):
                        hoff = (h % 2) * 64
                        qf, kf = h // 2, 4 + h // 2
                        pv = pvps.tile([P, 512], F32, tag="pv")
                        for i in range(kmax):
                            s_ps = sps.tile([P, 512], F32, tag="s")
                            nc.tensor.matmul(
                                s_ps,
                                qk[hoff : hoff + 64, kf, P * i : P * (i + 1)],
                                qk[hoff : hoff + 64, qf, qs],
                                start=True,
                                stop=True,
                            )
                            p_sb = ppool.tile([P, 512], F32R, tag="p")
                            nc.scalar.activation(p_sb, s_ps, EXP, scale=1.0 / 8.0)
                            if i >= 4 * j:
                                nc.vector.tensor_mul(p_sb, p_sb, masks[:, i - 4 * j, :])
                            nc.tensor.matmul(
                                pv[0:65, :],
                                vt[:, i, h, :],
                                p_sb,
                                start=(i == 0),
                                stop=(i == kmax - 1),
                            )
                        # normalize: r = 1/l broadcast over the 64 dh rows
                        nc.vector.reciprocal(r_all[64:65, :], pv[64:65, :])
                        r0 = tails.tile([1, 512], F32, tag="r0")
                        nc.sync.dma_start(r0, r_all[64:65, :])
                        rb = tails.tile([64, 512], F32, tag="rb")
                        nc.gpsimd.partition_broadcast(rb, r0, channels=64)
                        if h % 2 == 0:
                            nc.vector.tensor_mul(yt[0:64, qf, qs], pv[0:64, :], rb)
                        else:
                            ytmp = tails.tile([64, 512], F32R, tag="ytmp")
                            nc.vector.tensor_mul(ytmp, pv[0:64, :], rb)
                            nc.sync.dma_start(yt[64:128, qf, qs], ytmp)
                    # projection for this slab's tokens
                    for tt in range(4 * j, 4 * j + 4):
                        for e in range(2):
                            pp = projps.tile([P, 512], F32, tag="pj")
                            for c in range(HPC * DH // P):
                                nc.tensor.matmul(
                                    pp,
                                    yt[:, c, P * tt : P * (tt + 1)],
                                    wp[:, c, 512 * e : 512 * (e + 1)],
                                    start=(c == 0),
                                    stop=(c == HPC * DH // P - 1),
                                )
                            ob = outsb.tile([P, 512], F32, tag="ob")
                            nc.vector.tensor_copy(ob, pp)
                            nc.sync.dma_start(
                                out_d[P * tt : P * (tt + 1), 512 * e : 512 * (e + 1)], ob
                            )
        lp.__exit__(None, None, None)
    nc.compile()
    return nc


def _host_masks():
    m = np.zeros((P, 4, 512), dtype=np.float32)
    ql = np.arange(512)
    for p in range(4):
        for kl in range(P):
            m[kl, p, :] = (ql >= kl + P * p).astype(np.float32)
    return m


def kernel(x, w_attn, w_proj):
    global _cached_nc, LAST_EXEC_NS
    x = np.asarray(x, dtype=np.float32)
    w_attn = np.asarray(w_attn, dtype=np.float32)
    w_proj = np.asarray(w_proj, dtype=np.float32)

    if _cached_nc is None:
        _cached_nc = _build_program()
    nc = _cached_nc

    masks = _host_masks()
    in_maps = []
    for c in range(N_CORES):
        b, hg = c // 2, (c % 2) * HPC
        w_q = w_attn[hg * DH : hg * DH + HPC * DH, :]
        w_k = w_attn[D + hg * DH : D + hg * DH + HPC * DH, :]
        w_v = w_attn[2 * D + hg * DH : 2 * D + hg * DH + HPC * DH, :]
        in_maps.append(
            {
                "xt": np.ascontiguousarray(x[b].T),
                "wqk": np.ascontiguousarray(np.concatenate([w_q, w_k], axis=0).T),
                "wv": np.ascontiguousarray(w_v.T),
                "wp": np.ascontiguousarray(w_proj[:, hg * DH : hg * DH + HPC * DH].T),
                "masks": masks,
            }
        )

    res = run_bass_kernel_spmd(nc, in_maps, list(range(N_CORES)))
    LAST_EXEC_NS = res.exec_time_ns
    y = np.empty((B, T, D), dtype=np.float32)
    for b in range(B):
        y[b] = res.results[2 * b]["out"] + res.results[2 * b + 1]["out"]
    return y
